# revision 7
# baseline (speedup 1.0000x reference)
"""Patch-entropy (histogram binning) Trainium2 Bass kernel — v2 architecture.

Input x:[64,3,512,512] f32 -> out:[64,32,32] f32; data-parallel over 8 cores
(8 images per core), 2 groups of 4 images per core.

Per-core pipeline (per group g of 4 images):
  1. DMA x rows as [128=(img4,patchrow32), 512] f32 channel tiles.
  2. DVE gray: t=0.299R; t+=0.587G; t+=0.114B (exact reference fp32 order),
     y = min(t, 1-ulp)*32 written free-shuffled to (pc, r, c).
  3. DVE: mq = (y mod 1) - y = -trunc(y) = -q, exact integers in bf16.
  4. PE transposes [128,128] bf16 blocks -> pixel-on-partition layout
     Q[128=(r8,c16), 4096=(pc32,img4,pr32)] per half (r<8 / r>=8).
  5. DVE staircase masks: for t in 0..32: mask = (-q <= -t) [q >= t], bf16 4x.
  6. PE ones-matmul per 512-patch block sums each mask over the 128 pixels,
     accumulating halves in PSUM row (32*blk%4 + t): S_t per patch.
  7. Tail: counts c_t = S_t - S_{t+1} via partition-shift DMA + subtract;
     ScalarE Ln(c/256 + eps); ent = block-diag ones-matmul of -(c/256)*lnp;
     per-image min/max via free reduce + gpsimd partition all-reduce;
     normalize with per-partition scalar APs; one output DMA.
"""

import numpy as np
from contextlib import ExitStack

import concourse.bass as bass
import concourse.bacc as bacc
import concourse.tile as tile
import concourse.mybir as mybir

F32 = mybir.dt.float32
BF16 = mybir.dt.bfloat16
I16 = mybir.dt.int16
AO = mybir.AluOpType
AF = mybir.ActivationFunctionType

N_CORES = 8
IMG_PER_CORE = 8
C, H, W = 3, 512, 512
PS = 16            # patch size
NB = 32            # histogram bins
GRAY_W = (0.299, 0.587, 0.114)
EPS = 1e-8
ONE_LT1 = float(np.nextafter(np.float32(1.0), np.float32(0.0)))


def _build_body(ctx, tc, x, consts, out, n_img):
    nc = tc.nc
    n_grp = n_img // 4

    const_pool = ctx.enter_context(tc.tile_pool(name="const", bufs=1))
    ch_pool = ctx.enter_context(tc.tile_pool(name="ch", bufs=3))
    u_pool = ctx.enter_context(tc.tile_pool(name="u", bufs=2))
    mq_pool = ctx.enter_context(tc.tile_pool(name="mq", bufs=1))
    q_pool = ctx.enter_context(tc.tile_pool(name="q", bufs=1))
    eq_pool = ctx.enter_context(tc.tile_pool(name="eq", bufs=2))
    s_pool = ctx.enter_context(tc.tile_pool(name="s", bufs=2 * n_grp))
    z_pool = ctx.enter_context(tc.tile_pool(name="z", bufs=2))
    cnt_pool = ctx.enter_context(tc.tile_pool(name="cnt", bufs=2))
    tail_pool = ctx.enter_context(tc.tile_pool(name="tail", bufs=1))
    pst_pool = ctx.enter_context(tc.tile_pool(name="pst", bufs=2, space="PSUM"))
    sps_pool = ctx.enter_context(tc.tile_pool(name="sps", bufs=2, space="PSUM"))
    ent_ps_pool = ctx.enter_context(tc.tile_pool(name="entps", bufs=1, space="PSUM"))
    tail_ps_pool = ctx.enter_context(tc.tile_pool(name="tailps", bufs=1, space="PSUM"))

    # --- constants ---
    identF = const_pool.tile([128, 136], F32, tag="identF")
    nc.sync.dma_start(out=identF, in_=consts[:, :])
    ident = const_pool.tile([128, 128], BF16, tag="ident")
    nc.vector.tensor_copy(ident, identF[:, 0:128])
    ident8r = identF[:, 128:136]  # every 8-row block is eye(8)
    # ones63[:, 31] = 1, else 0; slice [31-t : 63-t] gives a [128,32] matrix
    # whose only nonzero column is t -> matmul drops the pixel-sum into
    # PSUM partition row t of a 32-row block.
    ones63 = const_pool.tile([128, 63], BF16, tag="ones63")
    nc.vector.memset(ones63, 0.0)
    nc.vector.memset(ones63[:, 31:32], 1.0)
    # obb[p, 4+j] = 1 for p in [32j, 32j+32); slice [4-4b : 12-4b] gives the
    # [128,8] block-diag lhsT whose block j sums into ent row 4b+j.
    n_t = 2 * n_grp
    obb = const_pool.tile([128, 12], F32, tag="obb")
    nc.vector.memset(obb, 0.0)
    for j in range(4):
        nc.vector.memset(obb[32 * j : 32 * (j + 1), 4 + j : 4 + j + 1], 1.0)
    eps_b = const_pool.tile([128, 1], F32, tag="eps_b")
    nc.vector.memset(eps_b, EPS)
    onesr = const_pool.tile([1, 8], F32, tag="onesr")
    nc.vector.memset(onesr, 1.0)

    x_r = x.rearrange("b c (pr r) w -> b c pr r w", r=PS)  # [n_img,3,32,16,512]

    S_tiles = []
    for g in range(n_grp):
        # --- gray + quantize: q = round(min(gray,1-ulp)*32 - 0.5) ==
        # trunc(min(gray,1-ulp)*32); HW f32->int16 rounds to nearest ---
        mq = mq_pool.tile([128, 8192], I16, tag="mq")
        mq_v = mq.rearrange("p (pc r c) -> p pc r c", pc=32, r=PS, c=PS)
        mq2 = mq_pool.tile([128, 8192], I16, tag="mq2")
        mq2_v = mq2.rearrange("p (pc r c) -> p pc r c", pc=32, r=PS, c=PS)
        for r in range(PS):
            xts = []
            for c in range(3):
                xt = ch_pool.tile([128, 512], F32, tag=f"x{c}")
                nc.sync.dma_start(out=xt[:], in_=x_r[4 * g : 4 * g + 4, c, :, r, :])
                xts.append(xt)
            t1 = u_pool.tile([128, 512], F32, tag="t1")
            nc.vector.tensor_scalar(t1, xts[0], GRAY_W[0], None,
                                    op0=AO.mult, op1=AO.bypass)
            t2 = u_pool.tile([128, 512], F32, tag="t2")
            nc.vector.scalar_tensor_tensor(t2, xts[1], GRAY_W[1], t1,
                                           op0=AO.mult, op1=AO.add)
            t3 = u_pool.tile([128, 512], F32, tag="t3")
            nc.vector.scalar_tensor_tensor(t3, xts[2], GRAY_W[2], t2,
                                           op0=AO.mult, op1=AO.add)
            u4 = u_pool.tile([128, 512], F32, tag="u4")
            nc.vector.tensor_scalar(u4, t3, ONE_LT1, None,
                                    op0=AO.min, op1=AO.bypass)
            # HW f32->int16 rounds to nearest; trunc(32u) ==
            # max(round(32u - 0.5), round(32u + 0.5) - 1) under any tie rule
            nc.vector.tensor_scalar(
                mq_v[:, :, r, :],
                u4.rearrange("p (pc c) -> p pc c", c=PS),
                32.0, -0.5, op0=AO.mult, op1=AO.add)
            nc.vector.tensor_scalar(
                mq2_v[:, :, r, :],
                u4.rearrange("p (pc c) -> p pc c", c=PS),
                32.0, 0.5, op0=AO.mult, op1=AO.add)

        # q = max(mq, mq2 - 1), int16 -> bf16 (exact; transpose needs bf16)
        mqb = mq_pool.tile([128, 8192], BF16, tag="mqb")
        nc.vector.scalar_tensor_tensor(mqb, mq2, -1.0, mq,
                                       op0=AO.add, op1=AO.max)
        mqb_v = mqb.rearrange("p (pc r c) -> p pc r c", pc=32, r=PS, c=PS)

        # --- PE transpose to pixel-on-partition ---
        Qs = []
        for h in range(2):
            Q = q_pool.tile([128, 4096], BF16, tag=f"Q{h}")
            for blk in range(8):
                pst = pst_pool.tile([128, 512], BF16, tag="pst")
                for k in range(4):
                    pc = blk * 4 + k
                    nc.tensor.transpose(
                        pst[:, 128 * k : 128 * (k + 1)],
                        mqb_v[:, pc, 8 * h : 8 * h + 8, :],
                        ident)
                nc.scalar.activation(Q[:, 512 * blk : 512 * (blk + 1)], pst, AF.Copy)
            Qs.append(Q)

        # --- staircase masks + PE sums ---
        S_ps0 = sps_pool.tile([128, 512], F32, tag="Sps")
        S_ps1 = sps_pool.tile([128, 512], F32, tag="Sps")
        S_ps = [S_ps0, S_ps1]
        for t in range(NB):
            ms = []
            for h in range(2):
                msk = eq_pool.tile([128, 4096], BF16, tag=f"msk{h}")
                nc.vector.tensor_scalar(msk, Qs[h], float(t), None,
                                        op0=AO.is_ge, op1=AO.bypass)
                ms.append(msk)
            lhs_t = ones63[:, 31 - t : 63 - t]
            for blk in range(8):
                bank, cc = divmod(blk, 4)
                o = S_ps[bank][32 * cc : 32 * cc + 32, :]
                sl = slice(512 * blk, 512 * (blk + 1))
                nc.tensor.matmul(o, lhs_t, ms[0][:, sl],
                                 start=(t == 0), stop=False,
                                 tile_position=(0, 32 * cc))
                nc.tensor.matmul(o, lhs_t, ms[1][:, sl],
                                 start=False, stop=(t == NB - 1),
                                 tile_position=(0, 32 * cc))
        for bank in range(2):
            S = s_pool.tile([128, 512], F32, tag="S")
            nc.scalar.activation(S, S_ps[bank], AF.Copy)
            S_tiles.append(S)

    # --- tail: counts, entropy ---
    # Group g's per-patch entropy lands at PSUM/SBUF partitions [32g, 32g+8)
    # (pcblk on rows) so every compute access starts at a 0/32 boundary.
    ent_ps = ent_ps_pool.tile([32 * (n_grp - 1) + 8, 512], F32, tag="entps")
    for i, S in enumerate(S_tiles):
        g, bank = divmod(i, 2)
        z = z_pool.tile([128, 512], F32, tag="z")
        nc.vector.memset(z, 0.0)
        for cc in range(4):
            nc.sync.dma_start(out=z[32 * cc : 32 * cc + 31, :],
                              in_=S[32 * cc + 1 : 32 * cc + 32, :])
        cnt = cnt_pool.tile([128, 512], F32, tag="cnt")
        nc.vector.tensor_sub(cnt, S, z)
        lnp = cnt_pool.tile([128, 512], F32, tag="lnp")
        nc.scalar.activation(lnp, cnt, AF.Ln, bias=eps_b[:, :], scale=1.0 / 256.0)
        prod = cnt_pool.tile([128, 512], F32, tag="prod")
        nc.vector.scalar_tensor_tensor(prod, cnt, -1.0 / 256.0, lnp,
                                       op0=AO.mult, op1=AO.mult)
        nc.tensor.matmul(ent_ps[32 * g : 32 * g + 8, :],
                         obb[:, 4 - 4 * bank : 12 - 4 * bank], prod,
                         start=(bank == 0), stop=(bank == 1),
                         tile_position=(0, 32 * g))

    np_ent = 32 * (n_grp - 1) + 8
    ent_sb = tail_pool.tile([np_ent, 512], F32, tag="ent")
    MXMN = tail_pool.tile([np_ent, 8], F32, tag="MXMN")
    MNt = tail_pool.tile([np_ent, 4], F32, tag="MNt")
    NR = tail_pool.tile([np_ent, 8], F32, tag="NR")
    norm = tail_pool.tile([np_ent, 512], F32, tag="norm")
    # free layouts: ent (k4, i4, pr32); norm (i4, pr32, k4) so the out DMA's
    # inner dim (pc chunk k) is contiguous on both sides.
    ent_v = ent_sb.rearrange("p (k i pr) -> p k i pr", k=4, i=4)
    norm_v = norm.rearrange("p (i pr k) -> p i pr k", i=4, pr=32)
    out_v = out.rearrange("b pr (pb k) -> b pb pr k", k=4)
    for g in range(n_grp):
        psl = slice(32 * g, 32 * g + 8)
        nc.scalar.activation(ent_sb[psl, :], ent_ps[psl, :], AF.Copy)
        # per-(partition,img) max into cols 0:4, negated min into cols 4:8
        for i in range(4):
            nc.vector.tensor_reduce(MXMN[psl, i : i + 1], ent_v[psl, :, i, :],
                                    axis=mybir.AxisListType.XY, op=AO.max)
            nc.vector.tensor_reduce(MNt[psl, i : i + 1], ent_v[psl, :, i, :],
                                    axis=mybir.AxisListType.XY, op=AO.min)
        nc.vector.tensor_scalar(MXMN[psl, 4:8], MNt[psl, :], -1.0, None,
                                op0=AO.mult, op1=AO.bypass)
        # combine over the 8 pcblk partitions: transpose -> free-reduce
        tp = tail_ps_pool.tile([8, 8], F32, tag="tp")
        nc.tensor.transpose(tp, MXMN[psl, :], ident8r[psl, :],
                            tile_position=(32 * g, 0))
        tp_sb = tail_pool.tile([8, 8], F32, tag=f"tpsb{g}")
        nc.scalar.activation(tp_sb, tp, AF.Copy)
        red = tail_pool.tile([8, 1], F32, tag=f"red{g}")
        nc.vector.tensor_reduce(red, tp_sb, axis=mybir.AxisListType.X,
                                op=AO.max)
        # rows 0:4 = per-img max, rows 4:8 = per-img -min; back to a row
        tq = tail_ps_pool.tile([1, 8], F32, tag="tq")
        nc.tensor.transpose(tq, red, ident8r[0:8, :], tile_position=(0, 0))
        row = tail_pool.tile([1, 8], F32, tag=f"row{g}")
        nc.scalar.activation(row, tq, AF.Copy)
        # bc row: cols 0:4 = -min, cols 4:8 = 1/(max - min + eps)
        bc = tail_pool.tile([1, 8], F32, tag=f"bc{g}")
        nc.vector.tensor_copy(bc[:, 0:4], row[:, 4:8])
        dnr = tail_pool.tile([1, 4], F32, tag=f"dnr{g}")
        nc.vector.scalar_tensor_tensor(dnr, row[:, 0:4], EPS, row[:, 4:8],
                                       op0=AO.add, op1=AO.add)
        nc.vector.reciprocal(bc[:, 4:8], dnr)
        # broadcast to the 8 pcblk partitions of this group
        bps = tail_ps_pool.tile([np_ent, 8], F32, tag="bps")
        nc.tensor.matmul(bps[psl, :], onesr, bc, start=True, stop=True,
                         tile_position=(0, 32 * g))
        nc.scalar.activation(NR[psl, :], bps[psl, :], AF.Copy)
        for i in range(4):
            nc.vector.tensor_scalar(
                norm_v[psl, i].rearrange("p pr k -> p k pr"),
                ent_v[psl, :, i, :],
                NR[psl, i : i + 1], NR[psl, 4 + i : 5 + i],
                op0=AO.add, op1=AO.mult)
            nc.sync.dma_start(out=out_v[4 * g + i],
                              in_=norm_v[psl, i])


def consts_np():
    a = np.zeros((128, 136), np.float32)
    a[:, 0:128] = np.eye(128, dtype=np.float32)
    for j in range(8):
        a[j::8, 128 + j] = 1.0
    return a


def build_program(n_img=IMG_PER_CORE):
    nc = bacc.Bacc(target_bir_lowering=True)
    x = nc.declare_dram_parameter("x", [n_img, C, H, W], F32, isOutput=False)
    consts = nc.declare_dram_parameter("consts", [128, 136], F32, isOutput=False)
    out = nc.declare_dram_parameter("out", [n_img, NB, NB], F32, isOutput=True)
    with tile.TileContext(nc) as tc:
        with ExitStack() as ctx:
            _build_body(ctx, tc, x[:], consts[:], out[:], n_img)
    return nc


_CACHED = {}


def _get_program(n_img):
    if n_img not in _CACHED:
        nc = build_program(n_img)
        nc.finalize()
        _CACHED[n_img] = nc
    return _CACHED[n_img]


def kernel(x, patch_size, num_bins):
    assert int(patch_size) == PS and int(num_bins) == NB
    x = np.asarray(x, dtype=np.float32)
    B = x.shape[0]
    assert x.shape == (B, C, H, W) and B % N_CORES == 0
    per = B // N_CORES
    nc = _get_program(per)

    cns = consts_np()
    in_maps = [
        {"x": x[i * per : (i + 1) * per], "consts": cns} for i in range(N_CORES)
    ]
    try:
        from concourse.bass_utils import run_bass_kernel_spmd

        res = run_bass_kernel_spmd(nc, in_maps, list(range(N_CORES)), trace=False)
        return np.concatenate(
            [res.results[i]["out"] for i in range(N_CORES)], axis=0
        )
    except Exception:
        from concourse.bass_interp import CoreSim

        outs = []
        for m in in_maps:
            sim = CoreSim(nc)
            for k, v in m.items():
                sim.tensor(k)[:] = v
            sim.simulate()
            outs.append(np.array(sim.tensor("out")))
        return np.concatenate(outs, axis=0)


if __name__ == "__main__":
    from reference import setup_inputs, reference

    inputs = {k: np.asarray(v) for k, v in setup_inputs().items()}
    expected = np.asarray(reference(**inputs))
    actual = kernel(**inputs)
    err = np.max(np.abs(actual - expected)) / max(1e-12, np.max(np.abs(expected)))
    print("Relative error:", err)
